# revision 23
# baseline (speedup 1.0000x reference)
"""Trainium2 Bass kernel for ConditionalLatentTrajectoryGenerator.

2-layer GRU rollout (B=128, T=512, H=1024, L=C=256) with FiLM conditioning
and an autoregressive linear head.

Sharding: data-parallel, batch 16 per core across 8 cores (weights replicated).

Per-core mapping: batch (16) is the stationary operand of every matmul
(lhsT = x.T [K,16]); weights are the moving operand, pre-permuted into 4
column-groups (tile_position col-tiling) so four weight streams run
concurrently on the PE array at the 4-column/cycle streaming roofline
(~9.6us/step of PE work). Weights live in SBUF in bf16. Per-example
constants enter PSUM via K=16 identity matmuls.

State h is kept striped (group g at partitions 32g..32g+16, hidden slice
[256g, 256g+256)). The x.T stationaries are refreshed each step with the
DVE 32x32 block transpose; the block-scrambled hidden order is absorbed
into the host-side weight row permutation.

v6 schedule, from NTFF trace analysis. Measured facts driving the design:
ACTIVATE ~474ns, DVE TT ~290ns (PSUM-src ~419), each matmul-stop ->
elementwise handoff ~330ns (drain + sem + dispatch), elementwise -> matmul
~250ns, and the Tile framework's RAW tracking is TILE-granular (a reader
waits for ALL writes to the tile, not just its slice). Hence:
  - every gate accumulator gets its OWN PSUM bank (8 banks: P1r P1z P1inn
    P1hn P2r P2z P2hn and P2inn sharing a bank with Pz), so r-gate sigmoids
    fire as soon as the r stream alone has stopped;
  - bank openers (start=True clears the whole bank): P1r/P1z/P1inn <- the
    per-example consts; P1hn <- gh1 hn k0; P2r/P2z/P2hn <- gh2 k0;
    P2inn <- gi2 inn k0; Pz <- cz const. The two shared-bank clears
    (gi2-inn k0 wiping Pz, cz wiping P2inn) land >2us after the other
    half's last reader by PE program order;
  - gi1 streams r,inn,z; gi2 streams r,inn,z with ACT order r2,tanh2,q2:
    layer 2 uses h2' = h2 + q2*(n2-h2) so only two DVE ops follow the late
    q2, and d2 = n2-h2 runs as soon as tanh2 lands;
  - yT = scalesT o h2T (scales pre-transposed on the host) so h2T comes
    off the critical path first and feeds gh2(u+1) early;
  - gh2(u) k0..2 streams after the previous head (covers the z feedback
    chain), k3..7 + consts fill the L1 window; gh1(u+1) streams between
    gi2 and the head;
  - U=16 steps per hardware-loop body: the For_i boundary runs a ~6.6us
    all-engine barrier + semaphore reset, amortized to ~0.4us/step.
"""

import os
import sys
import numpy as np

sys.path.insert(0, "/opt/trn_rl_repo")

import ml_dtypes  # noqa: E402
from concourse import bass, bacc, mybir, tile  # noqa: E402
from concourse import bass_utils  # noqa: E402

F32 = mybir.dt.float32
BF16 = mybir.dt.bfloat16
NPBF16 = ml_dtypes.bfloat16

H = 1024
L = 256
C = 256
B = 128
NCORES = 8
BC = B // NCORES  # 16 batch per core
G = 4             # column groups / stripes
HG = H // G       # 256 hidden per group
LG = L // G       # 64 latent cols per group
U = int(os.environ.get("K_U", "16"))  # steps per hardware-loop body
LAST_EXEC_NS = None
LAST_RESULT = None


def _striped_batch(x):
    """[BC, 4*S] -> [128, S] with stripe g at partitions 32g..32g+BC."""
    S = x.shape[1] // G
    out = np.zeros((128, S), dtype=x.dtype)
    for g in range(G):
        out[32 * g:32 * g + BC, :] = x[:, g * S:(g + 1) * S]
    return out


def _scrambledT(x):
    """[BC, K] -> [128, K//4] block-transposed layout.

    out[32g+j, 32c+b] = x[b, S*g + 32c + j], S = K//4 — matches what
    nc.vector.transpose produces from the striped batch layout.
    """
    K = x.shape[1]
    S = K // 4
    nch = S // 32
    out = np.zeros((128, 32 * nch), dtype=x.dtype)
    for g in range(G):
        for c in range(nch):
            blk = x[:, S * g + 32 * c:S * g + 32 * c + 32]  # [BC, 32]
            out[32 * g:32 * g + 32, 32 * c:32 * c + BC][:blk.shape[1], :] = blk.T
    return out


def _k_index(K):
    """kidx[c, p] = hidden index feeding moving-row p of K-chunk c."""
    S = K // 4
    nch = S // 32
    p = np.arange(128)
    return np.stack([S * (p // 32) + 32 * c + (p % 32) for c in range(nch)])


def _moving_weights(w):
    """w [rows, K] (rows already output-permuted) -> [128, nch*G*ncols] bf16.

    Column (c, g, j) at c*G*ncols + g*ncols + j holds w[g*ncols+j, kidx[c, p]]
    for partition p.
    """
    K = w.shape[1]
    kidx = _k_index(K)                      # [nch, 128]
    ncols = w.shape[0] // G
    sel = w.T[kidx]                         # [nch, 128, G*ncols]
    arr = sel.transpose(1, 0, 2).reshape(128, kidx.shape[0] * G * ncols)
    return np.ascontiguousarray(arr.astype(NPBF16))


def _const_cols(c, perm):
    return np.ascontiguousarray(c[:, perm].astype(NPBF16))


def _build_program(T, emit_hn1, emit_rz2, emit_inn2, emit_hn2,
                   use_hw_loop=True):
    nc = bacc.Bacc("TRN2", target_bir_lowering=False, debug=False,
                   num_devices=NCORES)

    def din(name, shape, dt):
        return nc.dram_tensor(name, list(shape), dt, kind="ExternalInput")

    # moving weights: every matrix split by gate (r / z / n) so each gate's
    # PSUM bank stops independently
    d_wa = {g: din(f"wa_{g}", [128, 2 * G * 256], BF16) for g in "rzn"}
    d_wb = {g: din(f"wb_{g}", [128, 8 * G * 256], BF16) for g in "rzn"}
    d_wc = {g: din(f"wc_{g}", [128, 8 * G * 256], BF16) for g in "rzn"}
    d_wd = {g: din(f"wd_{g}", [128, 8 * G * 256], BF16) for g in "rzn"}
    d_wh = din("wh", [128, 8 * G * LG], BF16)
    d_cr1 = din("cr1", [BC, G * 256], BF16)
    d_cz1 = din("cz1", [BC, G * 256], BF16)
    d_cinn1 = din("cinn1", [BC, G * 256], BF16)
    d_cz = din("cz", [BC, G * LG], BF16)
    d_chn1 = din("chn1", [BC, G * 256], BF16) if emit_hn1 else None
    d_crz2 = din("crz2", [BC, G * 512], BF16) if emit_rz2 else None
    d_cinn2 = din("cinn2", [BC, G * 256], BF16) if emit_inn2 else None
    d_chn2 = din("chn2", [BC, G * 256], BF16) if emit_hn2 else None
    d_scalesT = din("scalesT", [128, 256], BF16)
    d_ident = din("ident", [BC, BC], BF16)
    d_h1s = din("h1s0", [128, HG], BF16)
    d_h2s = din("h2s0", [128, HG], BF16)
    d_h1T = din("h1T0", [128, 256], BF16)
    d_h2T = din("h2T0", [128, 256], BF16)
    d_zT = din("zT0", [128, 64], BF16)

    d_out = nc.dram_tensor("out", [128, T * LG], F32, kind="ExternalOutput")

    def sb(name, shape, dt):
        return nc.alloc_sbuf_tensor(name, list(shape), dt)

    s_wa = {g: sb(f"s_wa_{g}", [128, 2 * G * 256], BF16) for g in "rzn"}
    s_wb = {g: sb(f"s_wb_{g}", [128, 8 * G * 256], BF16) for g in "rzn"}
    s_wc = {g: sb(f"s_wc_{g}", [128, 8 * G * 256], BF16) for g in "rzn"}
    s_wd = {g: sb(f"s_wd_{g}", [128, 8 * G * 256], BF16) for g in "rzn"}
    s_wh = sb("s_wh", [128, 8 * G * LG], BF16)
    s_cr1 = sb("s_cr1", [BC, G * 256], BF16)
    s_cz1 = sb("s_cz1", [BC, G * 256], BF16)
    s_cinn1 = sb("s_cinn1", [BC, G * 256], BF16)
    s_cz = sb("s_cz", [BC, G * LG], BF16)
    s_chn1 = sb("s_chn1", [BC, G * 256], BF16) if emit_hn1 else None
    s_crz2 = sb("s_crz2", [BC, G * 512], BF16) if emit_rz2 else None
    s_cinn2 = sb("s_cinn2", [BC, G * 256], BF16) if emit_inn2 else None
    s_chn2 = sb("s_chn2", [BC, G * 256], BF16) if emit_hn2 else None
    s_scalesT = sb("s_scalesT", [128, 256], BF16)
    s_ident = sb("s_ident", [BC, BC], BF16)
    s_h1s = [sb(f"s_h1s{i}", [128, HG], BF16) for i in range(2)]
    s_h2s = [sb(f"s_h2s{i}", [128, HG], BF16) for i in range(2)]
    s_h1T = [sb(f"s_h1T{i}", [128, 256], BF16) for i in range(2)]
    s_h2T = [sb(f"s_h2T{i}", [128, 256], BF16) for i in range(2)]
    s_zT = [sb(f"s_zT{i}", [128, 64], BF16) for i in range(2)]
    s_ring = sb("s_ring", [128, U * LG], F32)

    with tile.TileContext(nc) as tc:
        loads = [(s_wa[g], d_wa[g]) for g in "rzn"]
        loads += [(s_wb[g], d_wb[g]) for g in "rzn"]
        loads += [(s_wc[g], d_wc[g]) for g in "rzn"]
        loads += [(s_wd[g], d_wd[g]) for g in "rzn"]
        loads += [
            (s_wh, d_wh),
            (s_cr1, d_cr1), (s_cz1, d_cz1), (s_cinn1, d_cinn1), (s_cz, d_cz),
            (s_scalesT, d_scalesT),
            (s_ident, d_ident),
            (s_h1s[0], d_h1s), (s_h2s[0], d_h2s),
            (s_h1T[0], d_h1T), (s_h2T[0], d_h2T), (s_zT[0], d_zT),
        ]
        for s_opt, d_opt in ((s_chn1, d_chn1), (s_crz2, d_crz2),
                             (s_cinn2, d_cinn2), (s_chn2, d_chn2)):
            if s_opt is not None:
                loads.append((s_opt, d_opt))
        for s_t, d_t in loads:
            nc.sync.dma_start(s_t[:], d_t.ap())

        with tc.tile_pool(name="sp", bufs=2) as sp, \
             tc.tile_pool(name="pp", bufs=1, space="PSUM") as pp:

            # One PSUM bank per gate accumulator (RAW tracking is
            # tile-granular, so sharing a tile delays the early readers).
            # P2inn shares its bank with Pz: their openers' whole-bank
            # clears land >2us after the other half's last read (PE
            # program order), and the regions themselves don't overlap.
            P1r = pp.tile([128, 256], F32, tag="p1r", name="p1r")
            P1z = pp.tile([128, 256], F32, tag="p1z", name="p1z")
            P1inn = pp.tile([128, 256], F32, tag="p1inn", name="p1inn")
            P1hn = pp.tile([128, 256], F32, tag="p1hn", name="p1hn")
            P2r = pp.tile([128, 256], F32, tag="p2r", name="p2r")
            P2z = pp.tile([128, 256], F32, tag="p2z", name="p2z")
            P2iz = pp.tile([128, 512], F32, tag="p2iz", name="p2iz")
            P2hn = pp.tile([128, 256], F32, tag="p2hn", name="p2hn")
            P2inn = P2iz[:, 0:256]
            Pz = P2iz[:, 256:256 + LG]
            # Initialize the never-matmul-written garbage stripes once with a
            # full-partition zero matmul (only Matmult/Memset may write PSUM).
            s_zmm = sp.tile([16, 512], BF16, tag="zmm", name="s_zmm")
            nc.vector.memset(s_zmm[:], 0.0)
            for ptile, w in ((P1r, 256), (P1z, 256), (P1inn, 256),
                             (P1hn, 256), (P2r, 256), (P2z, 256),
                             (P2iz, 512), (P2hn, 256)):
                nc.tensor.matmul(ptile[:, 0:w], s_zmm[:, 0:128], s_zmm[:, 0:w],
                                 start=True, stop=True, skip_group_check=True)

            def mm(*a, **kw):
                nc.tensor.matmul(*a, skip_group_check=True, **kw)

            def lT(t, c):
                return t[:, 32 * c:32 * c + BC]

            Sig = mybir.ActivationFunctionType.Sigmoid
            Tanh = mybir.ActivationFunctionType.Tanh
            Copy = mybir.ActivationFunctionType.Copy

            def stream(ptile, width, stat, w, k, start, stop):
                for g in range(G):
                    mm(ptile[32 * g:32 * g + BC, :], lT(stat, k),
                       w[:, (k * G + g) * width:(k * G + g) * width + width],
                       start=start, stop=stop, tile_position=(0, 32 * g))

            def cstream(ptile, c, width, start, stop):
                for g in range(G):
                    mm(ptile[32 * g:32 * g + BC, :], s_ident[:],
                       c[:, g * width:g * width + width],
                       start=start, stop=stop, tile_position=(0, 32 * g))

            def emit_gi1(u):
                """gi1 r (stop), inn (stop), z (stop) — separate banks, so
                r1/t2/q1 fire off each gate's own stream end."""
                p = u % 2
                for gate, ptile in (("r", P1r), ("n", P1inn), ("z", P1z)):
                    for k in range(2):
                        stream(ptile, 256, s_zT[p], s_wa[gate], k,
                               start=False, stop=(k == 1))

            def emit_gh2(u, ks):
                """gh2(u) chunks ks; the k=0 matmuls open (bank-clear)
                P2r / P2z / P2hn; stops P2hn at k=7."""
                p = u % 2
                for k in ks:
                    stream(P2r, 256, s_h2T[p], s_wd["r"], k,
                           start=(k == 0), stop=False)
                    stream(P2z, 256, s_h2T[p], s_wd["z"], k,
                           start=(k == 0), stop=False)
                    stream(P2hn, 256, s_h2T[p], s_wd["n"], k,
                           start=(k == 0), stop=(k == 7 and not emit_hn2))
                if emit_hn2 and 7 in ks:
                    cstream(P2hn, s_chn2, 256, start=False, stop=True)

            def emit_consts():
                """Per-example consts open next step's P1r/P1z/P1inn."""
                cstream(P1r, s_cr1, 256, start=True, stop=False)
                cstream(P1z, s_cz1, 256, start=True, stop=False)
                cstream(P1inn, s_cinn1, 256, start=True, stop=False)

            def emit_gi2(u):
                """gi2(u): r k0..7 (stops P2r), inn (opens+stops P2inn),
                z (stops P2z)."""
                pw = (u + 1) % 2
                for k in range(8):
                    stream(P2r, 256, s_h1T[pw], s_wc["r"], k,
                           start=False, stop=(k == 7 and not emit_rz2))
                for k in range(8):
                    stream(P2inn, 256, s_h1T[pw], s_wc["n"], k,
                           start=(k == 0), stop=(k == 7 and not emit_inn2))
                for k in range(8):
                    stream(P2z, 256, s_h1T[pw], s_wc["z"], k,
                           start=False, stop=(k == 7 and not emit_rz2))
                if emit_rz2:
                    for g in range(G):
                        mm(P2r[32 * g:32 * g + BC, :], s_ident[:],
                           s_crz2[:, g * 512:g * 512 + 256],
                           start=False, stop=True, tile_position=(0, 32 * g))
                        mm(P2z[32 * g:32 * g + BC, :], s_ident[:],
                           s_crz2[:, g * 512 + 256:g * 512 + 512],
                           start=False, stop=True, tile_position=(0, 32 * g))
                if emit_inn2:
                    cstream(P2inn, s_cinn2, 256, start=False, stop=True)

            def emit_gh1(u, ks):
                """gh1(u) chunks ks: r/z accumulate (banks opened by the
                consts); hn k0 opens its own bank, k7 stops it."""
                p = u % 2
                for k in ks:
                    stream(P1r, 256, s_h1T[p], s_wb["r"], k,
                           start=False, stop=False)
                    stream(P1z, 256, s_h1T[p], s_wb["z"], k,
                           start=False, stop=False)
                    stream(P1hn, 256, s_h1T[p], s_wb["n"], k,
                           start=(k == 0), stop=(k == 7 and not emit_hn1))
                if emit_hn1 and 7 in ks:
                    cstream(P1hn, s_chn1, 256, start=False, stop=True)

            def emit_step(u):
                p, pw = u % 2, (u + 1) % 2

                # ---- PE: gi1 first so P1r stops earliest
                emit_gi1(u)
                # ---- PE: gh2(u) tail (k0..2 ran after the previous head)
                emit_gh2(u, range(3, 8))

                # ---- L1 elementwise (DVE FIFO = dataflow order)
                r1 = sp.tile([128, 256], BF16, tag="r1", name=f"r1_{u}")
                q1 = sp.tile([128, 256], BF16, tag="q1", name=f"q1_{u}")
                hnc = sp.tile([128, 256], BF16, tag="hnc", name=f"hnc_{u}")
                t1 = sp.tile([128, 256], BF16, tag="t1", name=f"t1_{u}")
                t2 = sp.tile([128, 256], BF16, tag="t2", name=f"t2_{u}")
                n1 = sp.tile([128, 256], BF16, tag="n1", name=f"n1_{u}")
                mm_ = sp.tile([128, 256], BF16, tag="mm1", name=f"mm1_{u}")
                m2 = sp.tile([128, 256], BF16, tag="m2", name=f"m2_{u}")
                m1 = sp.tile([128, 256], BF16, tag="m1", name=f"m1_{u}")
                # hn cast: P1hn's writes all landed last step, so this runs
                # at the step top in the DVE idle slot
                nc.vector.tensor_copy(hnc[:], P1hn[:])
                nc.scalar.activation(r1[:], P1r[:], Sig)
                nc.scalar.activation(q1[:], P1z[:], Sig, scale=-1.0)
                nc.vector.tensor_mul(t1[:], r1[:], hnc[:])
                nc.vector.tensor_add(t2[:], t1[:], P1inn[:])
                nc.scalar.activation(n1[:], t2[:], Tanh)
                # during tanh (q1 lands early here): m2 = h - q*h = z*h
                nc.vector.tensor_mul(mm_[:], q1[:], s_h1s[p][:])
                nc.vector.tensor_sub(m2[:], s_h1s[p][:], mm_[:])
                nc.vector.tensor_mul(m1[:], n1[:], q1[:])
                nc.vector.tensor_add(s_h1s[pw][:], m1[:], m2[:])
                nc.vector.transpose(s_h1T[pw][:, 0:128], s_h1s[pw][:, 0:128])
                nc.vector.transpose(s_h1T[pw][:, 128:256], s_h1s[pw][:, 128:256])

                # ---- PE: next step's const matmuls + gi2 + gh1
                emit_consts()
                emit_gi2(u)
                emit_gh1(u + 1, range(0, 8))

                # ---- L2 elementwise: h2' = h2 + q2*(n2 - h2) — only two
                # DVE ops follow the late q2 (its z stream ends last)
                r2 = sp.tile([128, 256], BF16, tag="r2", name=f"r2_{u}")
                q2 = sp.tile([128, 256], BF16, tag="q2", name=f"q2_{u}")
                hnc2 = sp.tile([128, 256], BF16, tag="hnc2", name=f"hnc2_{u}")
                t1b = sp.tile([128, 256], BF16, tag="t1b", name=f"t1b_{u}")
                t2b = sp.tile([128, 256], BF16, tag="t2b", name=f"t2b_{u}")
                n2 = sp.tile([128, 256], BF16, tag="n2", name=f"n2_{u}")
                d2 = sp.tile([128, 256], BF16, tag="d2", name=f"d2_{u}")
                qd2 = sp.tile([128, 256], BF16, tag="qd2", name=f"qd2_{u}")
                yT = sp.tile([128, 256], BF16, tag="yT", name=f"yT_{u}")
                nc.vector.tensor_copy(hnc2[:], P2hn[:])
                nc.scalar.activation(r2[:], P2r[:], Sig)
                nc.vector.tensor_mul(t1b[:], r2[:], hnc2[:])
                nc.vector.tensor_add(t2b[:], t1b[:], P2inn[:])
                nc.scalar.activation(n2[:], t2b[:], Tanh)
                nc.scalar.activation(q2[:], P2z[:], Sig, scale=-1.0)
                nc.vector.tensor_sub(d2[:], n2[:], s_h2s[p][:])
                nc.vector.tensor_mul(qd2[:], q2[:], d2[:])
                nc.vector.tensor_add(s_h2s[pw][:], s_h2s[p][:], qd2[:])
                # h2T first (feeds gh2(u+1)), then yT = scalesT o h2T
                nc.vector.transpose(s_h2T[pw][:, 0:128], s_h2s[pw][:, 0:128])
                nc.vector.transpose(s_h2T[pw][:, 128:256],
                                    s_h2s[pw][:, 128:256])
                nc.vector.tensor_mul(yT[:], s_scalesT[:], s_h2T[pw][:])

                # ---- PE: head (needs yT)
                cstream(Pz, s_cz, LG, start=True, stop=False)
                for k in range(8):
                    for g in range(G):
                        mm(Pz[32 * g:32 * g + BC, :], lT(yT, k),
                           s_wh[:, (k * G + g) * LG:(k * G + g) * LG + LG],
                           start=False, stop=(k == 7),
                           tile_position=(0, 32 * g))

                # ---- PE: gh2(u+1) k0..2 covers the z feedback chain
                emit_gh2(u + 1, range(0, 3))

                # ---- tail: z feedback (DVE cast + transpose) + output ring
                zb = sp.tile([128, LG], BF16, tag="zb", name=f"zb_{u}")
                nc.vector.tensor_copy(zb[:], Pz[:])
                nc.vector.transpose(s_zT[pw][:], zb[:])
                nc.scalar.activation(s_ring[:, u * LG:(u + 1) * LG], Pz[:],
                                     Copy)

            def emit_body(it_dma):
                for u in range(U):
                    emit_step(u)
                it_dma()

            # preamble: open P1 banks + gh1(0)/gh2(0) head chunks for the
            # first body iteration (in-loop, step U-1 emits them for the
            # next iteration)
            emit_consts()
            emit_gh1(0, range(0, 8))
            emit_gh2(0, range(0, 3))
            if use_hw_loop:
                with tc.For_i(0, T // U, 1,
                              hint_engines=(mybir.EngineType.PE,)) as it:
                    emit_body(lambda: nc.sync.dma_start(
                        d_out[:, bass.ts(it, U * LG)], s_ring[:]))
            else:
                for it in range(T // U):
                    emit_body(lambda it=it: nc.sync.dma_start(
                        d_out[:, it * U * LG:(it + 1) * U * LG], s_ring[:]))

    nc.compile()
    return nc


def kernel(z_start, cond_emb, max_len,
           z2h_w1, z2h_b1, z2h_w2, z2h_b2,
           w_ih1, w_hh1, b_ih1, b_hh1,
           w_ih2, w_hh2, b_ih2, b_hh2,
           film_w, film_b, head_w, head_b):
    z_start = np.asarray(z_start, np.float32)
    cond_emb = np.asarray(cond_emb, np.float32)
    T = int(max_len)
    assert T % U == 0
    f32 = lambda x: np.asarray(x, np.float32)
    w_ih1, w_hh1, b_ih1, b_hh1 = map(f32, (w_ih1, w_hh1, b_ih1, b_hh1))
    w_ih2, w_hh2, b_ih2, b_hh2 = map(f32, (w_ih2, w_hh2, b_ih2, b_hh2))
    film_w, film_b, head_w, head_b = map(f32, (film_w, film_b, head_w, head_b))
    z2h_w1, z2h_b1, z2h_w2, z2h_b2 = map(f32, (z2h_w1, z2h_b1, z2h_w2, z2h_b2))

    # ---------- host-side precompute ----------
    h0 = np.maximum(z_start @ z2h_w1.T + z2h_b1, 0.0) @ z2h_w2.T + z2h_b2
    film = cond_emb @ film_w.T + film_b
    gamma, beta = film[:, :H], film[:, H:]
    scale = 1.0 + gamma                      # [B, H]
    cz_full = beta @ head_w.T + head_b       # [B, L]
    gcond = cond_emb @ w_ih1[:, L:].T        # [B, 3H]
    crz1_full = gcond[:, :2 * H] + b_ih1[:2 * H] + b_hh1[:2 * H]
    cinn1_full = gcond[:, 2 * H:] + b_ih1[2 * H:]
    chn1_full = np.broadcast_to(b_hh1[2 * H:], (B, H)).copy()
    crz2_full = np.broadcast_to(b_ih2[:2 * H] + b_hh2[:2 * H], (B, 2 * H)).copy()
    cinn2_full = np.broadcast_to(b_ih2[2 * H:], (B, H)).copy()
    chn2_full = np.broadcast_to(b_hh2[2 * H:], (B, H)).copy()
    emit_hn1 = bool(np.any(chn1_full))
    emit_rz2 = bool(np.any(crz2_full))
    emit_inn2 = bool(np.any(cinn2_full))
    emit_hn2 = bool(np.any(chn2_full))

    # gate-row permutations into the striped (group, col) layout
    perm_r = np.concatenate([np.arange(HG * g, HG * g + HG)
                             for g in range(G)])              # rows of H
    perm_z = H + perm_r
    perm_n = 2 * H + perm_r
    perm_head = np.arange(L)
    cperm_rz = np.concatenate([
        np.concatenate([np.arange(HG * g, HG * g + HG),
                        H + np.arange(HG * g, HG * g + HG)])
        for g in range(G)])                                   # rows of 2H
    cperm_h = perm_r

    wz = w_ih1[:, :L]  # [3H, L] latent part
    weights = {}
    for name, wmat in (("wa", wz), ("wb", w_hh1), ("wc", w_ih2),
                       ("wd", w_hh2)):
        for gate, perm in (("r", perm_r), ("z", perm_z), ("n", perm_n)):
            weights[f"{name}_{gate}"] = _moving_weights(wmat[perm])
    wh = _moving_weights(head_w[perm_head])

    ident = np.eye(BC, dtype=NPBF16)

    use_hw_loop = os.environ.get("K_NO_HW_LOOP", "0") != "1"
    nc = _build_program(T, emit_hn1, emit_rz2, emit_inn2, emit_hn2,
                        use_hw_loop=use_hw_loop)

    in_maps = []
    for ci in range(NCORES):
        sl = slice(ci * BC, (ci + 1) * BC)
        m = dict(weights)
        m.update({
            "wh": wh, "ident": ident,
            "cr1": _const_cols(crz1_full[sl][:, :H], cperm_h),
            "cz1": _const_cols(crz1_full[sl][:, H:], cperm_h),
            "cinn1": _const_cols(cinn1_full[sl], cperm_h),
            "cz": _const_cols(cz_full[sl], perm_head),
            "scalesT": _scrambledT(scale[sl].astype(NPBF16)),
            "h1s0": _striped_batch(h0[sl].astype(NPBF16)),
            "h2s0": _striped_batch(h0[sl].astype(NPBF16)),
            "h1T0": _scrambledT(h0[sl].astype(NPBF16)),
            "h2T0": _scrambledT(h0[sl].astype(NPBF16)),
            "zT0": _scrambledT(z_start[sl].astype(NPBF16)),
        })
        if emit_hn1:
            m["chn1"] = _const_cols(chn1_full[sl], cperm_h)
        if emit_rz2:
            m["crz2"] = _const_cols(crz2_full[sl], cperm_rz)
        if emit_inn2:
            m["cinn2"] = _const_cols(cinn2_full[sl], cperm_h)
        if emit_hn2:
            m["chn2"] = _const_cols(chn2_full[sl], cperm_h)
        in_maps.append(m)

    trace = os.environ.get("K_TRACE", "0") == "1"
    res = bass_utils.run_bass_kernel_spmd(nc, in_maps,
                                          core_ids=list(range(NCORES)),
                                          trace=trace)
    global LAST_EXEC_NS, LAST_RESULT
    LAST_EXEC_NS = res.exec_time_ns
    LAST_RESULT = res

    out = np.empty((B, T, L), dtype=np.float32)
    for ci in range(NCORES):
        arr = res.results[ci]["out"].reshape(4, 32, T, LG)
        for g in range(G):
            out[ci * BC:(ci + 1) * BC, :, g * LG:(g + 1) * LG] = arr[g, :BC]
    return out
